# revision 6
# baseline (speedup 1.0000x reference)
"""Trainium2 Bass kernel for nn_CodeGNN (4-layer SAGE GNN + pool + fc + L2norm).

Strategy (8 NeuronCores, SPMD):
- Nodes are range-sharded by dst across cores (12500/core). Each core owns the
  aggregation for its nodes.
- Per layer, the "transform-first" form is used: z_l = h_l @ Wl_l is computed
  locally (feature-major matmul), transposed to node-major rows, and AllGathered
  into a replicated [TROWS, 64] f32 table in each core's HBM.
- The per-edge mean aggregation segment_sum(z[src], dst)/deg is done with:
  dma_gather (SWDGE, int16 idx, 3840 rows/call) -> edge-major SBUF tiles,
  one-hot selection matrices built on DVE (iota + is_equal), and PE matmuls
  (lhsT=gathered[128,64], rhs=onehot[128,80]) accumulating feature-major
  window sums in PSUM [64, 512] group accumulators.
- Static SPMD schedule: nodes are bin-packed into 80-node windows with
  per-pass (table-quarter) slot capacity so every PSUM column offset and
  gather size is a compile-time constant; all data-dependence lives in the
  per-core idx/dst_local input arrays. Pad slots gather row 0 of the quarter
  and carry dst_local=1e9 so their one-hot row is all zeros.
- Epilogue: graph mean-pool as a matmul against a host-built membership
  matrix (1/graph_count values), AllReduce over cores, fc, transpose, and
  per-graph L2 normalization.
"""
import sys

for _p in ("/opt/trn_rl_repo", "/root/.axon_site/_ro/trn_rl_repo"):
    if _p not in sys.path:
        sys.path.append(_p)

import numpy as np

P = 128
W = 80            # window width (PSUM columns per window)
WPG = 6           # windows per PSUM group (6*80=480 of 512 cols)
CPW = 5           # chunks per (window, pass)
NPASS = 4         # src-chunk passes (int16 table addressing)
IN_C, HID, OUT_C = 3, 64, 128
G_GRAPHS = 128
NCORES = 8
N_NODES = 100000
V_LOCAL = N_NODES // NCORES

CALL_CHUNKS = WPG * CPW            # 30
CALL_IDXS = CALL_CHUNKS * P        # 3840
WIN_SLOTS_PER_PASS = CPW * P       # 640


# ----------------------------------------------------------------------------
# host-side schedule construction
# ----------------------------------------------------------------------------

def _wrap_idxs(flat):
    n = flat.shape[0]
    blk = flat.reshape(n // 16, 16).T
    return np.tile(blk, (8, 1)).astype(np.int16)


def _pack_windows(deg_pass):
    V = deg_pass.shape[0]
    order = np.argsort(-deg_pass.sum(1), kind="stable")
    win_of = np.full(V, -1, np.int32)
    col_of = np.full(V, -1, np.int32)
    open_w = []
    n_win = 0
    cap = WIN_SLOTS_PER_PASS
    for node in order:
        need = deg_pass[node]
        placed = False
        for entry in open_w:
            rem, cnt, wid = entry
            if cnt < W and (need <= rem).all():
                win_of[node] = wid
                col_of[node] = cnt
                entry[0] = rem - need
                entry[1] = cnt + 1
                placed = True
                break
        if not placed:
            wid = n_win; n_win += 1
            open_w.insert(0, [np.full(NPASS, cap, np.int64) - need, 1, wid])
            win_of[node] = wid
            col_of[node] = 0
        if len(open_w) > 24:
            open_w.sort(key=lambda e: -int(e[0].sum()))
            open_w = open_w[:16]
    return win_of, col_of, n_win


def _prep(x, edge_index, batch):
    src = np.asarray(edge_index[0], np.int64)
    dst = np.asarray(edge_index[1], np.int64)
    batch = np.asarray(batch, np.int64)
    x = np.asarray(x, np.float32)

    deg = np.bincount(dst, minlength=N_NODES).astype(np.int64)
    inv_deg_full = 1.0 / np.maximum(deg, 1).astype(np.float32)
    gcount = np.bincount(batch, minlength=G_GRAPHS).astype(np.float32)
    inv_gcount = 1.0 / np.maximum(gcount, 1.0)

    core_of_dst = dst // V_LOCAL
    per_core = []
    n_windows = np.zeros(NCORES, np.int64)
    for c in range(NCORES):
        em = core_of_dst == c
        esrc, edst = src[em], dst[em]
        dloc = (edst - c * V_LOCAL).astype(np.int64)
        p_of_src = (esrc // V_LOCAL) // 2   # table quarter of the src row
        dp = np.zeros((V_LOCAL, NPASS), np.int64)
        np.add.at(dp, (dloc, p_of_src), 1)
        win_of, col_of, nw = _pack_windows(dp)
        n_windows[c] = nw
        per_core.append(dict(esrc=esrc, dloc=dloc, p_of_src=p_of_src,
                             win_of=win_of, col_of=col_of))

    NWIN = int(((n_windows.max() + WPG - 1) // WPG) * WPG)
    NGROUPS = NWIN // WPG
    NC_COLS = NGROUPS * 512
    TROWS = NCORES * NC_COLS
    QROWS = TROWS // NPASS
    assert QROWS <= 32767, f"QROWS={QROWS} exceeds int16 addressing"

    col_of_node = np.zeros(N_NODES, np.int64)
    for c in range(NCORES):
        pc = per_core[c]
        w, cl = pc["win_of"], pc["col_of"]
        col_of_node[c * V_LOCAL:(c + 1) * V_LOCAL] = \
            (w // WPG) * 512 + (w % WPG) * W + cl
    grow_of_node = (np.arange(N_NODES) // V_LOCAL) * NC_COLS + col_of_node

    NCHUNK = NGROUPS * NPASS * CALL_CHUNKS
    NCALLS = NGROUPS * NPASS
    out = dict(NGROUPS=NGROUPS, NC_COLS=NC_COLS, TROWS=TROWS, QROWS=QROWS,
               NCHUNK=NCHUNK, NCALLS=NCALLS, cores=[])

    for c in range(NCORES):
        pc = per_core[c]
        esrc, dloc = pc["esrc"], pc["dloc"]
        win_of, col_of = pc["win_of"], pc["col_of"]
        ew = win_of[dloc].astype(np.int64)
        ecol = col_of[dloc].astype(np.int64)
        ep = pc["p_of_src"].astype(np.int64)
        key = ((ew // WPG) * NPASS + ep) * WPG + (ew % WPG)
        order = np.argsort(key, kind="stable")
        se, ssrc, scol = key[order], esrc[order], ecol[order]
        uniq, first_idx = np.unique(se, return_index=True)
        seg_len = np.diff(np.append(first_idx, len(se)))
        pos_in_seg = np.arange(len(se)) - np.repeat(first_idx, seg_len)
        assert (pos_in_seg < WIN_SLOTS_PER_PASS).all(), "window-pass overflow"
        call_id = uniq // WPG
        wl = uniq % WPG
        slot_base = (call_id * CALL_CHUNKS + wl * CPW) * P
        slot = np.repeat(slot_base, seg_len) + pos_in_seg

        TOT_SLOTS = NCALLS * CALL_IDXS
        idx_flat = np.zeros(TOT_SLOTS, np.int64)
        dl_flat = np.full(TOT_SLOTS, 1e9, np.float32)
        idx_flat[slot] = grow_of_node[ssrc] % QROWS
        dl_flat[slot] = scol.astype(np.float32)

        idx_img = np.concatenate(
            [_wrap_idxs(idx_flat[k * CALL_IDXS:(k + 1) * CALL_IDXS].astype(np.int16))
             for k in range(NCALLS)], axis=1)
        dl_img = dl_flat.reshape(NCHUNK, P).T.copy()

        cols = (win_of.astype(np.int64) // WPG) * 512 + (win_of % WPG) * W + col_of
        invc = np.zeros(NC_COLS, np.float32)
        invc[cols] = inv_deg_full[c * V_LOCAL:(c + 1) * V_LOCAL]
        xT = np.zeros((IN_C, NC_COLS), np.float32)
        xT[:, cols] = x[c * V_LOCAL:(c + 1) * V_LOCAL].T
        mpool = np.zeros((NC_COLS, G_GRAPHS), np.float32)
        gb = batch[c * V_LOCAL:(c + 1) * V_LOCAL]
        mpool[cols, gb] = inv_gcount[gb]
        out["cores"].append(dict(idx=np.ascontiguousarray(idx_img),
                                 dl=np.ascontiguousarray(dl_img),
                                 invc=np.ascontiguousarray(invc[None, :]),
                                 xT=np.ascontiguousarray(xT),
                                 mpool=np.ascontiguousarray(mpool)))
    return out


# ----------------------------------------------------------------------------
# device program
# ----------------------------------------------------------------------------

_BUILD_CACHE = {}


def _build(NGROUPS):
    if NGROUPS in _BUILD_CACHE:
        return _BUILD_CACHE[NGROUPS]
    import concourse.bass as bass
    import concourse.bacc as bacc
    import concourse.mybir as mybir
    import concourse.tile as tile
    from concourse.masks import make_identity

    NC_COLS = NGROUPS * 512
    TROWS = NCORES * NC_COLS
    QROWS = TROWS // NPASS
    NCALLS = NGROUPS * NPASS
    NCHUNK = NCALLS * CALL_CHUNKS
    IDXW = CALL_IDXS // 16          # idx image cols per call (240)
    NPOOL = NC_COLS // P            # pooling chunks

    f32 = mybir.dt.float32
    i16 = mybir.dt.int16
    i32 = mybir.dt.int32

    nc = bacc.Bacc()
    d_xT = nc.declare_dram_parameter("xT", [IN_C, NC_COLS], f32, isOutput=False)
    d_idx = nc.declare_dram_parameter("idx", [P, NCALLS * IDXW], i16, isOutput=False)
    d_dl = nc.declare_dram_parameter("dl", [P, NCHUNK], f32, isOutput=False)
    d_invc = nc.declare_dram_parameter("invc", [1, NC_COLS], f32, isOutput=False)
    d_mpool = nc.declare_dram_parameter("mpool", [NC_COLS, G_GRAPHS], f32, isOutput=False)
    d_wl = [nc.declare_dram_parameter(f"wl{i}", [IN_C if i == 1 else HID, HID], f32,
                                      isOutput=False) for i in range(1, 5)]
    d_wr = [nc.declare_dram_parameter(f"wr{i}", [IN_C if i == 1 else HID, HID], f32,
                                      isOutput=False) for i in range(1, 5)]
    d_bl = [nc.declare_dram_parameter(f"bl{i}", [HID, 1], f32, isOutput=False)
            for i in range(1, 5)]
    d_wfc = nc.declare_dram_parameter("wfc", [HID, OUT_C], f32, isOutput=False)
    d_bfc = nc.declare_dram_parameter("bfc", [OUT_C, 1], f32, isOutput=False)
    d_out = nc.declare_dram_parameter("out", [G_GRAPHS, OUT_C], f32, isOutput=True)
    import os as _os
    KDEBUG = _os.environ.get("KDEBUG", "0") == "1"
    if KDEBUG:
        d_dbg_t = nc.declare_dram_parameter("dbg_table", [TROWS, HID], f32, isOutput=True)
        d_dbg_h = nc.declare_dram_parameter("dbg_h", [HID, NC_COLS], f32, isOutput=True)
        d_dbg_pagg = nc.declare_dram_parameter("dbg_pagg", [HID, 512], f32, isOutput=True)
        d_dbg_g = nc.declare_dram_parameter("dbg_g", [P, CALL_CHUNKS * HID], f32, isOutput=True)
        d_dbg_oh = nc.declare_dram_parameter("dbg_oh", [P, CALL_CHUNKS * W], f32, isOutput=True)

    with tile.TileContext(nc) as tc:
        with (
            tc.tile_pool(name="res", bufs=1) as res,
            tc.tile_pool(name="gp", bufs=3) as gp,
            tc.tile_pool(name="ohp", bufs=2) as ohp,
            tc.tile_pool(name="idxp", bufs=3) as idxp,
            tc.tile_pool(name="blk", bufs=3) as blk,
            tc.tile_pool(name="stg", bufs=2) as stg,
            tc.tile_pool(name="ps_agg", bufs=2, space="PSUM") as ps_agg,
            tc.tile_pool(name="ps_aux", bufs=2, space="PSUM") as ps_aux,
            tc.tile_pool(name="ps_tr", bufs=2, space="PSUM") as ps_tr,
            tc.tile_pool(name="dram", bufs=1, space="DRAM") as dram,
        ):
            # ---------------- resident tiles ----------------
            xT = res.tile([IN_C, NC_COLS], f32)
            nc.sync.dma_start(out=xT[:], in_=d_xT[:])
            dl = res.tile([P, NCHUNK], f32)
            nc.sync.dma_start(out=dl[:], in_=d_dl[:])
            iota_i = res.tile([P, W], i32)
            nc.gpsimd.iota(iota_i[:], pattern=[[1, W]], base=0, channel_multiplier=0)
            iota_f = res.tile([P, W], f32)
            nc.vector.tensor_copy(out=iota_f[:], in_=iota_i[:])
            id64 = res.tile([64, 64], f32)
            make_identity(nc, id64[:])
            id128 = res.tile([128, 128], f32)
            make_identity(nc, id128[:])
            wl = [res.tile([IN_C if i == 0 else HID, HID], f32, name=f"wlt{i}")
                  for i in range(4)]
            wr = [res.tile([IN_C if i == 0 else HID, HID], f32, name=f"wrt{i}")
                  for i in range(4)]
            bl = [res.tile([HID, 1], f32, name=f"blt{i}") for i in range(4)]
            for i in range(4):
                nc.sync.dma_start(out=wl[i][:], in_=d_wl[i][:])
                nc.sync.dma_start(out=wr[i][:], in_=d_wr[i][:])
                nc.sync.dma_start(out=bl[i][:], in_=d_bl[i][:])
            wfc = res.tile([HID, OUT_C], f32)
            nc.sync.dma_start(out=wfc[:], in_=d_wfc[:])
            bfc = res.tile([OUT_C, 1], f32)
            nc.sync.dma_start(out=bfc[:], in_=d_bfc[:])

            # ---------------- DRAM buffers ----------------
            tables = [dram.tile([TROWS, HID], f32, name=f"table{i}", bufs=1,
                                 addr_space="Shared")
                      for i in range(4)]
            cc_ins = [dram.tile([NC_COLS, HID], f32, name=f"ccin{i}", bufs=1)
                      for i in range(2)]
            h_dram = [dram.tile([HID, NC_COLS], f32, name=f"hdram{i}", bufs=1)
                      for i in range(2)]
            h4node = dram.tile([NC_COLS, HID], f32, bufs=1)
            ar_in = dram.tile([HID, G_GRAPHS], f32, bufs=1)
            ar_out = dram.tile([HID, G_GRAPHS], f32, bufs=1, addr_space="Shared")

            num_idxs_reg = nc.gpsimd.to_reg(CALL_IDXS)

            def stage_to_rows(zblk_t, dram_rows):
                """zblk_t [64, 512] feature-major -> node-major rows in dram."""
                stage = stg.tile([P, 4, HID], f32, tag="stage")
                for t in range(4):
                    ptr = ps_tr.tile([P, HID], f32, tag="ptr")
                    nc.tensor.transpose(ptr[:], zblk_t[:, t * P:(t + 1) * P], id64[:])
                    nc.vector.tensor_copy(out=stage[:, t, :], in_=ptr[:])
                nc.sync.dma_start(
                    out=dram_rows.rearrange("(t p) f -> p t f", p=P),
                    in_=stage[:])

            # ---------------- layer-0 table: z1 = x @ Wl1 ----------------
            for g in range(NGROUPS):
                pz = ps_aux.tile([HID, 512], f32, tag="pz")
                nc.tensor.matmul(pz[:], lhsT=wl[0][:], rhs=xT[:, g * 512:(g + 1) * 512],
                                 start=True, stop=True)
                zblk = blk.tile([HID, 512], f32, tag="zblk")
                nc.vector.tensor_copy(out=zblk[:], in_=pz[:])
                stage_to_rows(zblk[:], cc_ins[0][g * 512:(g + 1) * 512, :])
            nc.gpsimd.collective_compute(
                "AllGather", mybir.AluOpType.bypass,
                replica_groups=[list(range(NCORES))],
                ins=[cc_ins[0][:]], outs=[tables[0][:]])

            # ---------------- layers ----------------
            for l in range(4):
                table = tables[l]
                for g in range(NGROUPS):
                    pagg = ps_agg.tile([HID, 512], f32, tag="pagg")
                    for j in range(NPASS):
                        call = g * NPASS + j
                        idx_t = idxp.tile([P, IDXW], i16, tag="idx")
                        nc.sync.dma_start(
                            out=idx_t[:],
                            in_=d_idx[:, call * IDXW:(call + 1) * IDXW])
                        Gt = gp.tile([P, CALL_CHUNKS, HID], f32, tag="g")
                        nc.gpsimd.dma_gather(
                            Gt[:], table[j * QROWS:(j + 1) * QROWS, :], idx_t[:],
                            num_idxs=CALL_IDXS, num_idxs_reg=num_idxs_reg,
                            elem_size=HID, single_packet=False)
                        if KDEBUG and l == 0 and g == 0 and j == 0:
                            nc.sync.dma_start(out=d_dbg_g[:],
                                              in_=Gt[:].rearrange("p c f -> p (c f)"))
                        oh = ohp.tile([P, CALL_CHUNKS, W], f32, tag="oh")
                        dls = dl[:, call * CALL_CHUNKS:(call + 1) * CALL_CHUNKS]
                        nc.vector.tensor_tensor(
                            out=oh[:],
                            in0=dls.rearrange("p (c o) -> p c o", o=1)
                                   .to_broadcast([P, CALL_CHUNKS, W]),
                            in1=iota_f[:].rearrange("(p o) w -> p o w", o=1)
                                         .to_broadcast([P, CALL_CHUNKS, W]),
                            op=mybir.AluOpType.is_equal)
                        if KDEBUG and l == 0 and g == 0 and j == 0:
                            nc.sync.dma_start(out=d_dbg_oh[:],
                                              in_=oh[:].rearrange("p c w -> p (c w)"))
                        for k in range(CALL_CHUNKS):
                            wli = k // CPW
                            nc.tensor.matmul(
                                pagg[:, wli * W:wli * W + W],
                                lhsT=Gt[:, k, :], rhs=oh[:, k, :],
                                start=(j == 0 and k == 0),
                                stop=(j == NPASS - 1 and k == CALL_CHUNKS - 1),
                                skip_group_check=True)
                    if KDEBUG and l == 0 and g == 0:
                        dbgp = blk.tile([HID, 512], f32, tag="dbgp")
                        nc.vector.tensor_copy(out=dbgp[:], in_=pagg[:])
                        nc.sync.dma_start(out=d_dbg_pagg[:], in_=dbgp[:])
                    # ---- flush group g
                    invb = blk.tile([HID, 512], f32, tag="invb")
                    nc.sync.dma_start(
                        out=invb[:],
                        in_=d_invc[0:1, g * 512:(g + 1) * 512].to_broadcast([HID, 512]))
                    pself = ps_aux.tile([HID, 512], f32, tag="pz")
                    if l == 0:
                        nc.tensor.matmul(pself[:], lhsT=wr[0][:],
                                         rhs=xT[:, g * 512:(g + 1) * 512],
                                         start=True, stop=True)
                    else:
                        hprev = blk.tile([HID, 512], f32, tag="hprev")
                        nc.sync.dma_start(
                            out=hprev[:],
                            in_=h_dram[(l - 1) % 2][:, g * 512:(g + 1) * 512])
                        nc.tensor.matmul(pself[:], lhsT=wr[l][:], rhs=hprev[:],
                                         start=True, stop=True)
                    t1 = blk.tile([HID, 512], f32, tag="t1")
                    nc.vector.tensor_tensor(out=t1[:], in0=pagg[:], in1=invb[:],
                                            op=mybir.AluOpType.mult)
                    t2 = blk.tile([HID, 512], f32, tag="t2")
                    nc.vector.tensor_tensor(out=t2[:], in0=t1[:], in1=pself[:],
                                            op=mybir.AluOpType.add)
                    hn = blk.tile([HID, 512], f32, tag="hn")
                    nc.scalar.activation(hn[:], t2[:],
                                         mybir.ActivationFunctionType.Relu,
                                         bias=bl[l][:, :1])
                    nc.sync.dma_start(out=h_dram[l % 2][:, g * 512:(g + 1) * 512],
                                      in_=hn[:])
                    if l < 3:
                        pz = ps_aux.tile([HID, 512], f32, tag="pz")
                        nc.tensor.matmul(pz[:], lhsT=wl[l + 1][:], rhs=hn[:],
                                         start=True, stop=True)
                        zblk = blk.tile([HID, 512], f32, tag="zblk")
                        nc.vector.tensor_copy(out=zblk[:], in_=pz[:])
                        stage_to_rows(zblk[:], cc_ins[(l + 1) % 2][g * 512:(g + 1) * 512, :])
                    else:
                        stage_to_rows(hn[:], h4node[g * 512:(g + 1) * 512, :])
                if l < 3:
                    nc.gpsimd.collective_compute(
                        "AllGather", mybir.AluOpType.bypass,
                        replica_groups=[list(range(NCORES))],
                        ins=[cc_ins[(l + 1) % 2][:]], outs=[tables[l + 1][:]])

            if KDEBUG:
                nc.sync.dma_start(out=d_dbg_t[:], in_=tables[0][:])
                nc.sync.dma_start(out=d_dbg_h[:], in_=h_dram[0][:])

            # ---------------- pooling ----------------
            ppool = ps_tr.tile([HID, G_GRAPHS], f32, tag="ppool")
            for q in range(NPOOL):
                h4b = blk.tile([P, HID], f32, tag="h4b")
                nc.sync.dma_start(out=h4b[:], in_=h4node[q * P:(q + 1) * P, :])
                mpb = blk.tile([P, G_GRAPHS], f32, tag="mpb")
                nc.sync.dma_start(out=mpb[:], in_=d_mpool[q * P:(q + 1) * P, :])
                nc.tensor.matmul(ppool[:], lhsT=h4b[:], rhs=mpb[:],
                                 start=(q == 0), stop=(q == NPOOL - 1))
            pool_sb = res.tile([HID, G_GRAPHS], f32)
            nc.vector.tensor_copy(out=pool_sb[:], in_=ppool[:])
            nc.sync.dma_start(out=ar_in[:], in_=pool_sb[:])
            nc.gpsimd.collective_compute(
                "AllReduce", mybir.AluOpType.add,
                replica_groups=[list(range(NCORES))],
                ins=[ar_in[:]], outs=[ar_out[:]])
            pooled = res.tile([HID, G_GRAPHS], f32)
            nc.sync.dma_start(out=pooled[:], in_=ar_out[:])

            # ---------------- fc + normalize ----------------
            pfc = ps_aux.tile([OUT_C, G_GRAPHS], f32, tag="pz")
            nc.tensor.matmul(pfc[:], lhsT=wfc[:], rhs=pooled[:], start=True, stop=True)
            S = res.tile([OUT_C, G_GRAPHS], f32)
            nc.vector.tensor_scalar(out=S[:], in0=pfc[:], scalar1=bfc[:, :1],
                                    scalar2=None, op0=mybir.AluOpType.add)
            ptr2 = ps_tr.tile([G_GRAPHS, OUT_C], f32, tag="ptr")
            nc.tensor.transpose(ptr2[:], S[:], id128[:])
            Sg = res.tile([G_GRAPHS, OUT_C], f32)
            nc.vector.tensor_copy(out=Sg[:], in_=ptr2[:])
            sq = res.tile([G_GRAPHS, OUT_C], f32)
            nc.vector.tensor_tensor(out=sq[:], in0=Sg[:], in1=Sg[:],
                                    op=mybir.AluOpType.mult)
            ss = res.tile([G_GRAPHS, 1], f32)
            nc.vector.reduce_sum(out=ss[:], in_=sq[:], axis=mybir.AxisListType.X)
            nrm = res.tile([G_GRAPHS, 1], f32)
            nc.scalar.activation(nrm[:], ss[:], mybir.ActivationFunctionType.Sqrt)
            rinv = res.tile([G_GRAPHS, 1], f32)
            nc.vector.reciprocal(out=rinv[:], in_=nrm[:])
            outS = res.tile([G_GRAPHS, OUT_C], f32)
            nc.vector.tensor_scalar(out=outS[:], in0=Sg[:], scalar1=rinv[:, :1],
                                    scalar2=None, op0=mybir.AluOpType.mult)
            nc.sync.dma_start(out=d_out[:], in_=outS[:])

    nc.compile()
    _BUILD_CACHE[NGROUPS] = nc
    return nc


# ----------------------------------------------------------------------------
# entry point
# ----------------------------------------------------------------------------

def _run(inputs, trace=False):
    from concourse.bass_utils import run_bass_kernel_spmd
    pp = _prep(inputs["x"], inputs["edge_index"], inputs["batch"])
    nc = _build(pp["NGROUPS"])
    in_maps = []
    for c in range(NCORES):
        pc = pp["cores"][c]
        m = dict(xT=pc["xT"], idx=pc["idx"], dl=pc["dl"], invc=pc["invc"],
                 mpool=pc["mpool"])
        for i in range(1, 5):
            m[f"wl{i}"] = np.asarray(inputs[f"Wl{i}"], np.float32)
            m[f"wr{i}"] = np.asarray(inputs[f"Wr{i}"], np.float32)
            m[f"bl{i}"] = np.asarray(inputs[f"bl{i}"], np.float32).reshape(HID, 1)
        m["wfc"] = np.asarray(inputs["Wfc"], np.float32)
        m["bfc"] = np.asarray(inputs["bfc"], np.float32).reshape(OUT_C, 1)
        in_maps.append(m)
    res = run_bass_kernel_spmd(nc, in_maps, list(range(NCORES)), trace=trace)
    return res.results[0]["out"].astype(np.float32), res


def kernel(**inputs):
    out, _ = _run(inputs)
    return out


# revision 14
# speedup vs baseline: 2.1308x; 2.1308x over previous
"""Trainium2 Bass kernel for nn_CodeGNN (4-layer SAGE GNN + pool + fc + L2norm).

Strategy (8 NeuronCores, SPMD):
- Nodes range-sharded by dst across cores (12500/core); each core owns the
  aggregation for its nodes.
- Transform-first: z_l = h_l @ Wl_l computed locally feature-major, transposed
  to node-major rows, AllGathered into replicated HBM tables. Tables are split
  into 4 "quarters" by node residue (node % 4) so (a) int16 dma_gather
  indices can address each quarter, and (b) the 4 AllGathers fire as their
  quarter's producer groups finish, overlapping communication with the long
  gather pipeline of the same layer.
- Per-edge mean aggregation via dma_gather (SWDGE, 4 parallel queues) into
  edge-major SBUF tiles; one-hot selection built on DVE (iota + is_equal);
  PE matmuls (lhsT=gathered[128,64], rhs=onehot[128,80]) accumulate
  feature-major window sums in PSUM [64,512] group accumulators.
- Static SPMD schedule: per quarter, nodes are bin-packed into 80-node
  windows with a 640-slot cap per (window, src-residue); all PSUM offsets and
  gather sizes are compile-time constants; data-dependence lives in the
  per-core idx / dst_local input arrays. Pad slots gather row 0 and carry
  dst_local=1e9 so their one-hot row is zero.
- Graph mean-pooling is fused into layer 3 (matmul of transposed h4 blocks
  against a host-built 1/graph_count membership matrix), then AllReduce, fc,
  transpose and per-graph L2 normalization.
"""
import sys

for _p in ("/opt/trn_rl_repo", "/root/.axon_site/_ro/trn_rl_repo"):
    if _p not in sys.path:
        sys.path.append(_p)

import numpy as np

P = 128
W = 80            # window width (PSUM columns per window)
WPG = 6           # max windows per PSUM group (6*80=480 of 512 cols)
CPW = 5           # chunks per (window, pass)
NPASS = 4         # node residue classes = table quarters
IN_C, HID, OUT_C = 3, 64, 128
G_GRAPHS = 128
NCORES = 8
N_NODES = 100000
V_LOCAL = N_NODES // NCORES

WIN_SLOTS = CPW * P               # 640 slots per (window, pass)


def _wrap_idxs(flat):
    n = flat.shape[0]
    blk = flat.reshape(n // 16, 16).T
    return np.tile(blk, (8, 1)).astype(np.int16)


def _quarter_layout(NWQ):
    """Group sizes (windows per group) within one quarter."""
    fullg = NWQ // WPG
    stub = NWQ - fullg * WPG
    return [WPG] * fullg + ([stub] if stub else [])


def _pack_windows(deg_pass):
    V = deg_pass.shape[0]
    order = np.argsort(-deg_pass.sum(1), kind="stable")
    win_of = np.full(V, -1, np.int32)
    col_of = np.full(V, -1, np.int32)
    open_w = []
    n_win = 0
    for node in order:
        need = deg_pass[node]
        placed = False
        for entry in open_w:
            rem, cnt, wid = entry
            if cnt < W and (need <= rem).all():
                win_of[node] = wid
                col_of[node] = cnt
                entry[0] = rem - need
                entry[1] = cnt + 1
                placed = True
                break
        if not placed:
            wid = n_win; n_win += 1
            open_w.insert(0, [np.full(NPASS, WIN_SLOTS, np.int64) - need, 1, wid])
            win_of[node] = wid
            col_of[node] = 0
        if len(open_w) > 24:
            open_w.sort(key=lambda e: -int(e[0].sum()))
            open_w = open_w[:16]
    return win_of, col_of, n_win


def _prep(x, edge_index, batch):
    src = np.asarray(edge_index[0], np.int64)
    dst = np.asarray(edge_index[1], np.int64)
    batch = np.asarray(batch, np.int64)
    x = np.asarray(x, np.float32)

    deg = np.bincount(dst, minlength=N_NODES).astype(np.int64)
    inv_deg_full = 1.0 / np.maximum(deg, 1).astype(np.float32)
    gcount = np.bincount(batch, minlength=G_GRAPHS).astype(np.float32)
    inv_gcount = 1.0 / np.maximum(gcount, 1.0)

    core_of_dst = dst // V_LOCAL
    e_pass = (src % NPASS).astype(np.int64)   # table quarter of an edge's src

    per_core = []
    nwq_all = np.zeros((NCORES, NPASS), np.int64)
    for c in range(NCORES):
        em = core_of_dst == c
        esrc, edst = src[em], dst[em]
        dloc = (edst - c * V_LOCAL).astype(np.int64)
        ep = e_pass[em]
        dp = np.zeros((V_LOCAL, NPASS), np.int64)
        np.add.at(dp, (dloc, ep), 1)
        # pack each dst-residue class separately (node residue = dloc % 4
        # since core base offsets are multiples of 4)
        win_of = np.full(V_LOCAL, -1, np.int32)
        col_of = np.full(V_LOCAL, -1, np.int32)
        for q in range(NPASS):
            sel = np.where((np.arange(V_LOCAL) % NPASS) == q)[0]
            wq, cq, nw = _pack_windows(dp[sel])
            win_of[sel] = wq
            col_of[sel] = cq
            nwq_all[c, q] = nw
        per_core.append(dict(esrc=esrc, dloc=dloc, ep=ep,
                             win_of=win_of, col_of=col_of))

    NWQ = int(nwq_all.max())
    sizes = _quarter_layout(NWQ)
    NGQ = len(sizes)
    NGROUPS = NPASS * NGQ
    NC_COLS = NGROUPS * 512
    ROWS_Q = NWQ * W                  # compact table rows per rank per quarter
    TQROWS = NCORES * ROWS_Q          # rows of one quarter table
    assert TQROWS <= 32767, f"TQROWS={TQROWS} exceeds int16"

    # window-in-quarter -> (group-in-quarter, window-in-group)
    w2gi = np.zeros(NWQ, np.int64)
    w2wl = np.zeros(NWQ, np.int64)
    wi = 0
    for gi, sz in enumerate(sizes):
        for wli in range(sz):
            w2gi[wi] = gi; w2wl[wi] = wli; wi += 1

    col_of_node = np.zeros(N_NODES, np.int64)   # 512-stride column layout
    trow_of_node = np.zeros(N_NODES, np.int64)  # compact row within (rank, q)
    for c in range(NCORES):
        pc = per_core[c]
        nq = np.arange(V_LOCAL) % NPASS
        gi = w2gi[pc["win_of"]]
        wli = w2wl[pc["win_of"]]
        grp = nq * NGQ + gi
        col_of_node[c * V_LOCAL:(c + 1) * V_LOCAL] = grp * 512 + wli * W + pc["col_of"]
        trow_of_node[c * V_LOCAL:(c + 1) * V_LOCAL] = pc["win_of"] * W + pc["col_of"]

    rank_of_node = np.arange(N_NODES) // V_LOCAL
    qrow_of_node = rank_of_node * ROWS_Q + trow_of_node   # row in quarter table

    chunks_of_group = [sizes[g % NGQ] * CPW for g in range(NGROUPS)]
    NCHUNK = sum(cg * NPASS for cg in chunks_of_group)
    call_chunk_base = {}
    ch = 0
    for g in range(NGROUPS):
        for j in range(NPASS):
            call_chunk_base[(g, j)] = ch
            ch += chunks_of_group[g]
    TOT_SLOTS = NCHUNK * P

    out = dict(NWQ=NWQ, NGQ=NGQ, NGROUPS=NGROUPS, NC_COLS=NC_COLS,
               ROWS_Q=ROWS_Q, TQROWS=TQROWS, NCHUNK=NCHUNK, cores=[])

    for c in range(NCORES):
        pc = per_core[c]
        esrc, dloc, ep = pc["esrc"], pc["dloc"], pc["ep"]
        win_of, col_of = pc["win_of"], pc["col_of"]
        nq = dloc % NPASS
        ew = win_of[dloc].astype(np.int64)
        eg = nq * NGQ + w2gi[ew]
        ewl = w2wl[ew]
        ecol = col_of[dloc].astype(np.int64)

        key = (eg * NPASS + ep) * WPG + ewl
        order = np.argsort(key, kind="stable")
        se, ssrc, scol = key[order], esrc[order], ecol[order]
        uniq, first_idx = np.unique(se, return_index=True)
        seg_len = np.diff(np.append(first_idx, len(se)))
        pos = np.arange(len(se)) - np.repeat(first_idx, seg_len)
        assert (pos < WIN_SLOTS).all(), "window-pass overflow"
        u_g = uniq // (NPASS * WPG)
        u_j = (uniq // WPG) % NPASS
        u_wl = uniq % WPG
        seg_base = np.array([(call_chunk_base[(g, j)] + wl * CPW) * P
                             for g, j, wl in zip(u_g, u_j, u_wl)])
        slot = np.repeat(seg_base, seg_len) + pos

        idx_flat = np.zeros(TOT_SLOTS, np.int64)
        dl_flat = np.full(TOT_SLOTS, 1e9, np.float32)
        idx_flat[slot] = qrow_of_node[ssrc]
        dl_flat[slot] = scol.astype(np.float32)

        pieces = []
        for g in range(NGROUPS):
            for j in range(NPASS):
                b = call_chunk_base[(g, j)] * P
                n = chunks_of_group[g] * P
                pieces.append(_wrap_idxs(idx_flat[b:b + n].astype(np.int16)))
        idx_img = np.concatenate(pieces, axis=1)
        dl_img = dl_flat.reshape(NCHUNK, P).T.copy()

        cols = col_of_node[c * V_LOCAL:(c + 1) * V_LOCAL]
        invc = np.zeros(NC_COLS, np.float32)
        invc[cols] = inv_deg_full[c * V_LOCAL:(c + 1) * V_LOCAL]
        xT = np.zeros((IN_C, NC_COLS), np.float32)
        xT[:, cols] = x[c * V_LOCAL:(c + 1) * V_LOCAL].T
        # pooling membership in compact table-row layout [4*ROWS_Q, 128]
        mpool = np.zeros((NPASS * ROWS_Q, G_GRAPHS), np.float32)
        gb = batch[c * V_LOCAL:(c + 1) * V_LOCAL]
        nqv = np.arange(V_LOCAL) % NPASS
        mrow = nqv * ROWS_Q + trow_of_node[c * V_LOCAL:(c + 1) * V_LOCAL]
        mpool[mrow, gb] = inv_gcount[gb]
        out["cores"].append(dict(idx=np.ascontiguousarray(idx_img),
                                 dl=np.ascontiguousarray(dl_img),
                                 invc=np.ascontiguousarray(invc[None, :]),
                                 xT=np.ascontiguousarray(xT),
                                 mpool=np.ascontiguousarray(mpool)))
    return out


# ----------------------------------------------------------------------------
# device program
# ----------------------------------------------------------------------------

_BUILD_CACHE = {}


def _build(NWQ):
    if NWQ in _BUILD_CACHE:
        return _BUILD_CACHE[NWQ]
    import concourse.bass as bass
    import concourse.bacc as bacc
    import concourse.mybir as mybir
    import concourse.tile as tile
    from concourse.masks import make_identity

    sizes = _quarter_layout(NWQ)
    NGQ = len(sizes)
    NGROUPS = NPASS * NGQ
    NC_COLS = NGROUPS * 512
    ROWS_Q = NWQ * W
    TQROWS = NCORES * ROWS_Q
    chunks_of_group = [sizes[g % NGQ] * CPW for g in range(NGROUPS)]
    NCHUNK = sum(cg * NPASS for cg in chunks_of_group)
    MAXCH = WPG * CPW

    f32 = mybir.dt.float32
    i16 = mybir.dt.int16
    i32 = mybir.dt.int32

    nc = bacc.Bacc(num_swdge_queues=4)
    d_xT = nc.declare_dram_parameter("xT", [IN_C, NC_COLS], f32, isOutput=False)
    d_idx = nc.declare_dram_parameter("idx", [P, NCHUNK * 8], i16, isOutput=False)
    d_dl = nc.declare_dram_parameter("dl", [P, NCHUNK], f32, isOutput=False)
    d_invc = nc.declare_dram_parameter("invc", [1, NC_COLS], f32, isOutput=False)
    d_mpool = nc.declare_dram_parameter("mpool", [NPASS * ROWS_Q, G_GRAPHS], f32,
                                        isOutput=False)
    d_wl = [nc.declare_dram_parameter(f"wl{i}", [IN_C if i == 1 else HID, HID], f32,
                                      isOutput=False) for i in range(1, 5)]
    d_wr = [nc.declare_dram_parameter(f"wr{i}", [IN_C if i == 1 else HID, HID], f32,
                                      isOutput=False) for i in range(1, 5)]
    d_bl = [nc.declare_dram_parameter(f"bl{i}", [HID, 1], f32, isOutput=False)
            for i in range(1, 5)]
    d_wfc = nc.declare_dram_parameter("wfc", [HID, OUT_C], f32, isOutput=False)
    d_bfc = nc.declare_dram_parameter("bfc", [OUT_C, 1], f32, isOutput=False)
    d_out = nc.declare_dram_parameter("out", [G_GRAPHS, OUT_C], f32, isOutput=True)

    with tile.TileContext(nc) as tc:
        with (
            tc.tile_pool(name="res", bufs=1) as res,
            tc.tile_pool(name="gp", bufs=5) as gp,
            tc.tile_pool(name="ohp", bufs=3) as ohp,
            tc.tile_pool(name="idxp", bufs=4) as idxp,
            tc.tile_pool(name="blk", bufs=2) as blk,
            tc.tile_pool(name="stg", bufs=2) as stg,
            tc.tile_pool(name="ps_agg", bufs=2, space="PSUM") as ps_agg,
            tc.tile_pool(name="ps_aux", bufs=2, space="PSUM") as ps_aux,
            tc.tile_pool(name="ps_tr", bufs=2, space="PSUM") as ps_tr,
            tc.tile_pool(name="ps_pool", bufs=1, space="PSUM") as ps_pool,
            tc.tile_pool(name="dram", bufs=1, space="DRAM") as dram,
        ):
            # ---------------- resident tiles ----------------
            xT = res.tile([IN_C, NC_COLS], f32)
            nc.sync.dma_start(out=xT[:], in_=d_xT[:])
            dl = res.tile([P, NCHUNK], f32)
            nc.sync.dma_start(out=dl[:], in_=d_dl[:])
            iota_i = res.tile([P, W], i32)
            nc.gpsimd.iota(iota_i[:], pattern=[[1, W]], base=0, channel_multiplier=0)
            iota_f = res.tile([P, W], f32)
            nc.vector.tensor_copy(out=iota_f[:], in_=iota_i[:])
            id64 = res.tile([64, 64], f32)
            make_identity(nc, id64[:])
            id128 = res.tile([128, 128], f32)
            make_identity(nc, id128[:])
            wl = [res.tile([IN_C if i == 0 else HID, HID], f32, name=f"wlt{i}")
                  for i in range(4)]
            wr = [res.tile([IN_C if i == 0 else HID, HID], f32, name=f"wrt{i}")
                  for i in range(4)]
            bl = [res.tile([HID, 1], f32, name=f"blt{i}") for i in range(4)]
            for i in range(4):
                nc.sync.dma_start(out=wl[i][:], in_=d_wl[i][:])
                nc.sync.dma_start(out=wr[i][:], in_=d_wr[i][:])
                nc.sync.dma_start(out=bl[i][:], in_=d_bl[i][:])
            wfc = res.tile([HID, OUT_C], f32)
            nc.sync.dma_start(out=wfc[:], in_=d_wfc[:])
            bfc = res.tile([OUT_C, 1], f32)
            nc.sync.dma_start(out=bfc[:], in_=d_bfc[:])

            # ---------------- DRAM buffers ----------------
            # quarter tables with 2-parity reuse (Local: multi-writer allowed)
            tables = [[dram.tile([TQROWS, HID], f32, name=f"table{p_}_{q}", bufs=1)
                       for q in range(NPASS)] for p_ in range(2)]
            cc_ins = [[dram.tile([ROWS_Q, HID], f32, name=f"ccin{p_}_{q}", bufs=1)
                       for q in range(NPASS)] for p_ in range(2)]
            h_dram = [dram.tile([HID, NC_COLS], f32, name=f"hdram{i}", bufs=1)
                      for i in range(2)]
            ar_in = dram.tile([HID, G_GRAPHS], f32, bufs=1)
            ar_out = dram.tile([HID, G_GRAPHS], f32, bufs=1, addr_space="Shared")

            reg_full = nc.gpsimd.to_reg(MAXCH * P)
            stub_ch = sizes[-1] * CPW
            reg_stub = nc.gpsimd.to_reg(stub_ch * P)

            call_chunk_base = {}
            ch = 0
            for g in range(NGROUPS):
                for j in range(NPASS):
                    call_chunk_base[(g, j)] = ch
                    ch += chunks_of_group[g]

            ppool = ps_pool.tile([HID, G_GRAPHS], f32)

            def stage_group(src_t, g, par, do_dma=True):
                """src_t [64, 512] feature-major -> compact node-major rows of
                cc_ins[par][quarter]; returns the stage tile."""
                q, gq = g // NGQ, g % NGQ
                nwin = sizes[gq]
                stage = stg.tile([W, WPG, HID], f32, tag="stage")
                for wli in range(nwin):
                    ptr = ps_tr.tile([W, HID], f32, tag="ptr")
                    nc.tensor.transpose(ptr[:], src_t[:, wli * W:(wli + 1) * W], id64[:])
                    nc.vector.tensor_copy(out=stage[:, wli, :], in_=ptr[:])
                if do_dma:
                    row0 = gq * WPG * W
                    nc.sync.dma_start(
                        out=cc_ins[par][q][row0:row0 + nwin * W, :]
                            .rearrange("(w p) f -> p w f", p=W),
                        in_=stage[:, :nwin, :])
                return stage

            def pool_group(stage, g):
                q, gq = g // NGQ, g % NGQ
                nwin = sizes[gq]
                for wli in range(nwin):
                    grow = q * ROWS_Q + (gq * WPG + wli) * W
                    mpb = blk.tile([W, G_GRAPHS], f32, tag="mpb")
                    nc.sync.dma_start(out=mpb[:], in_=d_mpool[grow:grow + W, :])
                    nc.tensor.matmul(ppool[:], lhsT=stage[:, wli, :], rhs=mpb[:],
                                     start=(g == 0 and wli == 0),
                                     stop=(g == NGROUPS - 1 and wli == nwin - 1),
                                     skip_group_check=True)

            # ---------------- layer-0 tables: z1 = x @ Wl1 ----------------
            for g in range(NGROUPS):
                pz = ps_aux.tile([HID, 512], f32, tag="pz")
                nc.tensor.matmul(pz[:], lhsT=wl[0][:], rhs=xT[:, g * 512:(g + 1) * 512],
                                 start=True, stop=True)
                zblk = blk.tile([HID, 512], f32, tag="zblk")
                nc.vector.tensor_copy(out=zblk[:], in_=pz[:])
                stage_group(zblk[:], g, 0)
                if (g + 1) % NGQ == 0:
                    q = g // NGQ
                    nc.gpsimd.collective_compute(
                        "AllGather", mybir.AluOpType.bypass,
                        replica_groups=[list(range(NCORES))],
                        ins=[cc_ins[0][q][:]], outs=[tables[0][q][:]])

            # ---------------- layers ----------------
            for l in range(4):
                par, npar = l % 2, (l + 1) % 2
                for g in range(NGROUPS):
                    nch = chunks_of_group[g]
                    pagg = ps_agg.tile([HID, 512], f32, tag="pagg")
                    idx_g = idxp.tile([P, MAXCH * NPASS * 8], i16, tag="idx")
                    ib = call_chunk_base[(g, 0)] * 8
                    nc.sync.dma_start(
                        out=idx_g[:, :nch * NPASS * 8],
                        in_=d_idx[:, ib:ib + nch * NPASS * 8])
                    for j in range(NPASS):
                        Gt = gp.tile([P, MAXCH, HID], f32, tag="g")
                        nc.gpsimd.dma_gather(
                            Gt[:, :nch, :], tables[par][j][:],
                            idx_g[:, j * nch * 8:(j + 1) * nch * 8],
                            num_idxs=nch * P,
                            num_idxs_reg=(reg_full if nch == MAXCH else reg_stub),
                            elem_size=HID, single_packet=False, queue_num=j)
                        oh = ohp.tile([P, MAXCH, W], f32, tag="oh")
                        cb = call_chunk_base[(g, j)]
                        dls = dl[:, cb:cb + nch]
                        nc.vector.tensor_tensor(
                            out=oh[:, :nch, :],
                            in0=dls.rearrange("p (c o) -> p c o", o=1)
                                   .to_broadcast([P, nch, W]),
                            in1=iota_f[:].rearrange("(p o) w -> p o w", o=1)
                                         .to_broadcast([P, nch, W]),
                            op=mybir.AluOpType.is_equal)
                        for k in range(nch):
                            wli = k // CPW
                            nc.tensor.matmul(
                                pagg[:, wli * W:wli * W + W],
                                lhsT=Gt[:, k, :], rhs=oh[:, k, :],
                                start=(j == 0 and k == 0),
                                stop=(j == NPASS - 1 and k == nch - 1),
                                skip_group_check=True)
                    # ---- flush group g
                    invb = blk.tile([HID, 512], f32, tag="invb")
                    nc.sync.dma_start(
                        out=invb[:],
                        in_=d_invc[0:1, g * 512:(g + 1) * 512].to_broadcast([HID, 512]))
                    pself = ps_aux.tile([HID, 512], f32, tag="pz")
                    if l == 0:
                        nc.tensor.matmul(pself[:], lhsT=wr[0][:],
                                         rhs=xT[:, g * 512:(g + 1) * 512],
                                         start=True, stop=True)
                    else:
                        hprev = blk.tile([HID, 512], f32, tag="hprev")
                        nc.sync.dma_start(
                            out=hprev[:],
                            in_=h_dram[(l - 1) % 2][:, g * 512:(g + 1) * 512])
                        nc.tensor.matmul(pself[:], lhsT=wr[l][:], rhs=hprev[:],
                                         start=True, stop=True)
                    t1 = blk.tile([HID, 512], f32, tag="t1")
                    nc.vector.tensor_tensor(out=t1[:], in0=pagg[:], in1=invb[:],
                                            op=mybir.AluOpType.mult)
                    t2 = blk.tile([HID, 512], f32, tag="t2")
                    nc.vector.tensor_tensor(out=t2[:], in0=t1[:], in1=pself[:],
                                            op=mybir.AluOpType.add)
                    hn = blk.tile([HID, 512], f32, tag="hn")
                    nc.scalar.activation(hn[:], t2[:],
                                         mybir.ActivationFunctionType.Relu,
                                         bias=bl[l][:, :1])
                    if l < 3:
                        nc.sync.dma_start(out=h_dram[l % 2][:, g * 512:(g + 1) * 512],
                                          in_=hn[:])
                        pz = ps_aux.tile([HID, 512], f32, tag="pz")
                        nc.tensor.matmul(pz[:], lhsT=wl[l + 1][:], rhs=hn[:],
                                         start=True, stop=True)
                        zblk = blk.tile([HID, 512], f32, tag="zblk")
                        nc.vector.tensor_copy(out=zblk[:], in_=pz[:])
                        stage_group(zblk[:], g, npar)
                        if (g + 1) % NGQ == 0:
                            q = g // NGQ
                            nc.gpsimd.collective_compute(
                                "AllGather", mybir.AluOpType.bypass,
                                replica_groups=[list(range(NCORES))],
                                ins=[cc_ins[npar][q][:]], outs=[tables[npar][q][:]])
                    else:
                        stage = stage_group(hn[:], g, npar, do_dma=False)
                        pool_group(stage, g)

            # ---------------- pooled -> AllReduce -> fc -> normalize --------
            pool_sb = res.tile([HID, G_GRAPHS], f32)
            nc.vector.tensor_copy(out=pool_sb[:], in_=ppool[:])
            nc.sync.dma_start(out=ar_in[:], in_=pool_sb[:])
            nc.gpsimd.collective_compute(
                "AllReduce", mybir.AluOpType.add,
                replica_groups=[list(range(NCORES))],
                ins=[ar_in[:]], outs=[ar_out[:]])
            pooled = res.tile([HID, G_GRAPHS], f32)
            nc.sync.dma_start(out=pooled[:], in_=ar_out[:])

            pfc = ps_aux.tile([OUT_C, G_GRAPHS], f32, tag="pz")
            nc.tensor.matmul(pfc[:], lhsT=wfc[:], rhs=pooled[:], start=True, stop=True)
            S = res.tile([OUT_C, G_GRAPHS], f32)
            nc.vector.tensor_scalar(out=S[:], in0=pfc[:], scalar1=bfc[:, :1],
                                    scalar2=None, op0=mybir.AluOpType.add)
            ptr2 = ps_tr.tile([G_GRAPHS, OUT_C], f32, tag="ptr")
            nc.tensor.transpose(ptr2[:], S[:], id128[:])
            Sg = res.tile([G_GRAPHS, OUT_C], f32)
            nc.vector.tensor_copy(out=Sg[:], in_=ptr2[:])
            sq = res.tile([G_GRAPHS, OUT_C], f32)
            nc.vector.tensor_tensor(out=sq[:], in0=Sg[:], in1=Sg[:],
                                    op=mybir.AluOpType.mult)
            ss = res.tile([G_GRAPHS, 1], f32)
            nc.vector.reduce_sum(out=ss[:], in_=sq[:], axis=mybir.AxisListType.X)
            nrm = res.tile([G_GRAPHS, 1], f32)
            nc.scalar.activation(nrm[:], ss[:], mybir.ActivationFunctionType.Sqrt)
            rinv = res.tile([G_GRAPHS, 1], f32)
            nc.vector.reciprocal(out=rinv[:], in_=nrm[:])
            outS = res.tile([G_GRAPHS, OUT_C], f32)
            nc.vector.tensor_scalar(out=outS[:], in0=Sg[:], scalar1=rinv[:, :1],
                                    scalar2=None, op0=mybir.AluOpType.mult)
            nc.sync.dma_start(out=d_out[:], in_=outS[:])

    nc.compile()
    _BUILD_CACHE[NWQ] = nc
    return nc


# ----------------------------------------------------------------------------
# entry point
# ----------------------------------------------------------------------------

def _run(inputs, trace=False):
    from concourse.bass_utils import run_bass_kernel_spmd
    pp = _prep(inputs["x"], inputs["edge_index"], inputs["batch"])
    nc = _build(pp["NWQ"])
    in_maps = []
    for c in range(NCORES):
        pc = pp["cores"][c]
        m = dict(xT=pc["xT"], idx=pc["idx"], dl=pc["dl"], invc=pc["invc"],
                 mpool=pc["mpool"])
        for i in range(1, 5):
            m[f"wl{i}"] = np.asarray(inputs[f"Wl{i}"], np.float32)
            m[f"wr{i}"] = np.asarray(inputs[f"Wr{i}"], np.float32)
            m[f"bl{i}"] = np.asarray(inputs[f"bl{i}"], np.float32).reshape(HID, 1)
        m["wfc"] = np.asarray(inputs["Wfc"], np.float32)
        m["bfc"] = np.asarray(inputs["bfc"], np.float32).reshape(OUT_C, 1)
        in_maps.append(m)
    res = run_bass_kernel_spmd(nc, in_maps, list(range(NCORES)), trace=trace)
    return res.results[0]["out"].astype(np.float32), res


def kernel(**inputs):
    out, _ = _run(inputs)
    return out
